# revision 11
# baseline (speedup 1.0000x reference)
"""Neural CDE (RK4, piecewise-constant path derivative) Trainium2 kernel.

v2: fp16 3-term matmuls + 2 decoupled batch streams per core.

Strategy: B=128 -> 16 per core across 8 cores; each core runs TWO independent
streams of 8 samples so their dependency chains interleave across engines.
State is feature-major "split form": [128, 8] per stream whose semantic value
is top[64] + bottom[64] (L1 weights row-duplicated fold the halves for free).

Matmuls run in double-fp16: W ~= Wh + Wr (fp16 value + fp16 residual of the
fp32 weight) and activations h ~= hh + hr likewise. Keeping three terms
Wh.hh + Wh.hr + Wr.hh leaves a per-step arithmetic error ~(5e-4)^2 = 2.4e-7
(the chaotic dynamics amplify per-step error ~3000x over 512 steps, so
anything coarser - single bf16/fp16, 2-3-term bf16, RK3 - fails the 2e-2
budget; measured rel err here is ~3.2e-4).

L1/L2 accumulate bias (K=2 seed matmul) + all three terms into one psum
region; relu reads psum directly (Act, fp16 out) and the residual is one
DVE stt: hr = max(psum,0) - hh. L3 packs [hh|hr] as one N=16 moving tile
per chunk (Wh) plus an N=8 accumulate (Wr), seeds b3h/b3r per half via a
[16,128] one-hot matmul, and folds the halves with a pair-stride
tensor_reduce from psum. Tail per stream: tanh (Act) -> elementwise *
[dt*v | -1e-3*dt*sum v] (GpSimd) -> strided reduce (DVE) -> state update
(DVE stt for the fp32 slot; GpSimd copy/subtract for the fp16 zh/zr pair).
GpSimd (Pool) can only run tensor_tensor/copy/memset-class ops and cannot
touch PSUM; everything else is balanced across DVE and Act accordingly.
"""

import os
import sys
from contextlib import ExitStack

import numpy as np
import ml_dtypes

sys.path.insert(0, "/opt/trn_rl_repo")

import concourse.bass as bass
import concourse.tile as tile
from concourse import bacc
from concourse import mybir
from concourse.bass_utils import run_bass_kernel_spmd

B, L, X, Z, H = 128, 512, 16, 64, 128
NCORES = 8
BPC = B // NCORES   # 16 samples per core
SPS = BPC // 2      # 8 samples per stream
DT = 0.1
F32 = mybir.dt.float32
F16 = mybir.dt.float16
AF = mybir.ActivationFunctionType
OP = mybir.AluOpType

# x-major permutation: psum position (p, c) holds original W3 column z*16+x
# with x = 2c + (p>=64), z = p%64
_p = np.arange(128)
_c = np.arange(8)
ORIG_COL = (_p[None, :] % 64) * 16 + 2 * _c[:, None] + (_p[None, :] // 64)  # [8,128]


def build_nc(l_steps=L):
    nc = bacc.Bacc("TRN2")

    dp = nc.declare_dram_parameter
    vsmall = dp("vsmall", [l_steps, 256], F32, isOutput=False).ap()  # dt*v x-major
    svd = dp("svd", [l_steps, 16], F32, isOutput=False).ap()  # -1e-3*dt*sum_x v
    # [w1h|w1r|w2h|w2r|w3h(1024)|w3r(1024)] fp16, single DMA
    wmm_d = dp("wmm", [128, 2560], F16, isOutput=False).ap()
    # [seedW(128) | sel16(128)] fp16: seedW = [b3h;b3r] rows, sel16 one-hot
    seed_d = dp("seed", [16, 256], F16, isOutput=False).ap()
    # bias rows: [2,256] fp16 = [b1h;b1r | b2h;b2r]; ones2 [2,16] fp16
    biasw_d = dp("biasw", [2, 272], F16, isOutput=False).ap()
    wi1x_d = dp("wi1x", [16, 144], F32, isOutput=False).ap()  # [wi1 | x0t]
    wi2_d = dp("wi2", [128, 128], F32, isOutput=False).ap()
    wi3_d = dp("wi3", [128, 64], F32, isOutput=False).ap()
    bi1_d = dp("bi1", [128, 1], F32, isOutput=False).ap()
    bi2_d = dp("bi2", [128, 1], F32, isOutput=False).ap()
    bi3_d = dp("bi3", [64, 1], F32, isOutput=False).ap()
    # split-form state per step per stream; host folds top+bottom halves
    zallA = dp("zallA", [l_steps, 128, SPS], F32, isOutput=True).ap()
    zallB = dp("zallB", [l_steps, 128, SPS], F32, isOutput=True).ap()

    with tile.TileContext(nc) as tc, ExitStack() as ctx:
        singles = ctx.enter_context(tc.tile_pool(name="singles", bufs=1))
        wfp = ctx.enter_context(tc.tile_pool(name="wfp", bufs=4))
        gep = ctx.enter_context(tc.tile_pool(name="gep", bufs=4))
        gfp = ctx.enter_context(tc.tile_pool(name="gfp", bufs=2))
        hp = ctx.enter_context(tc.tile_pool(name="hp", bufs=3))
        ap_ = ctx.enter_context(tc.tile_pool(name="ap", bufs=3))
        zp = ctx.enter_context(tc.tile_pool(name="zp", bufs=3))
        mp = ctx.enter_context(tc.tile_pool(name="mp", bufs=2))
        qp = ctx.enter_context(tc.tile_pool(name="qp", bufs=4))
        kp = ctx.enter_context(tc.tile_pool(name="kp", bufs=4))
        php = ctx.enter_context(tc.tile_pool(name="php", bufs=2, space="PSUM"))
        gpp = ctx.enter_context(tc.tile_pool(name="gpp", bufs=2, space="PSUM"))

        dma = nc.sync.dma_start

        def load(pool, ap):
            t = pool.tile(list(ap.shape), ap.dtype, tag=ap.tensor.name)
            dma(out=t[:], in_=ap)
            return t

        wmm = load(singles, wmm_d)
        w1h, w1r = wmm[:, 0:128], wmm[:, 128:256]
        w2h, w2r = wmm[:, 256:384], wmm[:, 384:512]
        w3h, w3r = wmm[:, 512:1536], wmm[:, 1536:2560]
        seedt = load(singles, seed_d)
        seedW, sel16 = seedt[:, 0:128], seedt[:, 128:256]
        biasw = load(singles, biasw_d)
        b1w, b2w, ones2 = biasw[:, 0:128], biasw[:, 128:256], biasw[:, 256:264]
        wi1x = load(singles, wi1x_d)
        wi1, x0t = wi1x[:, 0:128], wi1x[:, 128:144]
        wi2 = load(singles, wi2_d)
        wi3 = load(singles, wi3_d)
        bi1 = load(singles, bi1_d)
        bi2 = load(singles, bi2_d)
        bi3 = load(singles, bi3_d)

        mm = nc.tensor.matmul

        # vector-engine handles per stream: A -> DVE, B -> GpSimd
        VE = [nc.vector, nc.gpsimd]
        SE = [nc.gpsimd, nc.vector]  # state-update engines, crossed

        # ---- init MLP (fp32): z0 = mlp(x(t0)), all 16 samples at once ----
        ph_i1 = php.tile([128, 32], F32, tag="phA")
        mm(ph_i1[:, 0:16], wi1, x0t, start=True, stop=True, skip_group_check=True)
        hi1 = singles.tile([128, BPC], F32, tag="hi1")
        nc.scalar.activation(hi1[:], ph_i1[:, 0:16], AF.Relu, bias=bi1[:])
        ph_i2 = php.tile([128, 32], F32, tag="phB")
        mm(ph_i2[:, 0:16], wi2[:], hi1[:], start=True, stop=True,
           skip_group_check=True)
        hi2 = singles.tile([128, BPC], F32, tag="hi2")
        nc.scalar.activation(hi2[:], ph_i2[:, 0:16], AF.Relu, bias=bi2[:])
        ph_i3 = php.tile([128, 32], F32, tag="phA")
        mm(ph_i3[0:64, 0:16], wi3[:], hi2[:], start=True, stop=True,
           skip_group_check=True)

        # per-stream state tiles for step 0
        ge_cur = [None, None]   # [128, 72] f32: [g(64) | slot(8)]
        zs_cur = [None, None]   # [128, 16] f16: [zh | zr]
        for S in range(2):
            ge = gep.tile([128, 72], F32, tag=f"ge{S}")
            ve = nc.vector
            ve.tensor_scalar_add(ge[0:64, 64:72],
                                 ph_i3[0:64, 8 * S:8 * S + SPS], bi3[:])
            ve.memset(ge[64:128, 64:72], 0.0)
            zs = zp.tile([128, 16], F16, tag=f"zs{S}")
            ve.tensor_copy(out=zs[:, 0:8], in_=ge[:, 64:72])
            ve.scalar_tensor_tensor(
                out=zs[:, 8:16], in0=zs[:, 0:8], scalar=-1.0, in1=ge[:, 64:72],
                op0=OP.mult, op1=OP.add,
            )  # DVE: init only
            ge_cur[S] = ge
            zs_cur[S] = zs

        stage_scale = [0.5, 0.5, 1.0]

        def apv(base, off, dims):
            # view with the tile's own partition dim (stride = row pitch)
            return bass.AP(tensor=base.tensor, offset=base.offset + off,
                           ap=[list(base.ap[0])] + dims)

        for t in range(l_steps):
            # wf [128, 144]: cols 0:128 dt*v replicated (c-major blocks of 16 j),
            # cols 128:144 svd broadcast; shared by both streams (j-sliced views)
            wf = wfp.tile([128, 144], F32, tag="wf")
            vbase = vsmall[t]
            for half in range(2):
                src = bass.AP(
                    tensor=vbase.tensor,
                    offset=vbase.offset + 16 * half,
                    ap=[[0, 64], [32, 8], [1, 16]],
                )
                dst = wf[64 * half:64 * (half + 1), 0:128].rearrange(
                    "p (c j) -> p c j", j=16
                )
                dma(out=dst, in_=src)
            sbase = svd[t]
            src = bass.AP(tensor=sbase.tensor, offset=sbase.offset,
                          ap=[[0, 128], [1, 16]])
            dma(out=wf[:, 128:144], in_=src)

            # output: split-form state at start of step t (host folds halves)
            nc.gpsimd.dma_start(out=zallA[t], in_=ge_cur[0][:, 64:72])
            nc.gpsimd.dma_start(out=zallB[t], in_=ge_cur[1][:, 64:72])

            ge_s, zs_s = list(ge_cur), list(zs_cur)
            qs = [[], []]
            kacc12 = [None, None]
            kacc123 = [None, None]
            pfin = [None, None]
            ge_next = [None, None]
            zs_next = [None, None]

            for s in range(4):
                ph = [None, None]
                av = [None, None]
                h1 = [None, None]
                # --- L1: psum[0:8] = Wh.zh + Wr.zh ; psum[8:16] = Wh.zr ---
                for S in range(2):
                    p = php.tile([128, 32], F32, tag=f"ph{'AB'[S]}")
                    mm(p[:, 0:8], b1w, ones2, start=True, stop=False,
                       skip_group_check=True)
                    mm(p[:, 0:8], w1h, zs_s[S][:, 0:8], start=False, stop=False,
                       skip_group_check=True)
                    mm(p[:, 0:8], w1h, zs_s[S][:, 8:16], start=False, stop=False,
                       skip_group_check=True)
                    mm(p[:, 0:8], w1r, zs_s[S][:, 0:8], start=False, stop=True,
                       skip_group_check=True)
                    ph[S] = p
                # relu -> hh (f16, Act, direct from psum); hr = max(ps,0) - hh
                for S in range(2):
                    h = hp.tile([128, 16], F16, tag=f"h1{'AB'[S]}")
                    nc.scalar.activation(h[:, 0:8], ph[S][:, 0:8], AF.Relu,
                                         bias=0.0)
                    h1[S] = h
                for S in range(2):
                    nc.vector.scalar_tensor_tensor(
                        out=h1[S][:, 8:16], in0=ph[S][:, 0:8], scalar=0.0,
                        in1=h1[S][:, 0:8], op0=OP.max, op1=OP.subtract,
                    )
                # --- L2 ---
                av2 = [None, None]
                h2 = [None, None]
                for S in range(2):
                    p = ph[S]
                    mm(p[:, 8:16], b2w, ones2, start=True, stop=False,
                       skip_group_check=True)
                    mm(p[:, 8:16], w2h, h1[S][:, 0:8], start=False, stop=False,
                       skip_group_check=True)
                    mm(p[:, 8:16], w2h, h1[S][:, 8:16], start=False, stop=False,
                       skip_group_check=True)
                    mm(p[:, 8:16], w2r, h1[S][:, 0:8], start=False, stop=True,
                       skip_group_check=True)
                for S in range(2):
                    h = hp.tile([128, 16], F16, tag=f"h2{'AB'[S]}")
                    nc.scalar.activation(h[:, 0:8], ph[S][:, 8:16], AF.Relu,
                                         bias=0.0)
                    h2[S] = h
                for S in range(2):
                    nc.vector.scalar_tensor_tensor(
                        out=h2[S][:, 8:16], in0=ph[S][:, 8:16], scalar=0.0,
                        in1=h2[S][:, 0:8], op0=OP.max, op1=OP.subtract,
                    )
                # --- L3: seed b3 pair, then 8 chunks ---
                gp = [None, None]
                for S in range(2):
                    g = gpp.tile([128, 128], F32, tag=f"gp{'AB'[S]}")
                    # seed: psum[c,0:8] = b3h[c], psum[c,8:16] = b3r[c]
                    mm(g[:], seedW, sel16, start=True, stop=False,
                       skip_group_check=True)
                    for c in range(8):
                        whc = w3h[:, c * 128:(c + 1) * 128]
                        wrc = w3r[:, c * 128:(c + 1) * 128]
                        sl = g[:, c * 16:(c + 1) * 16]
                        mm(sl, whc, h2[S][:], start=False, stop=False,
                           skip_group_check=True)
                        mm(sl[:, 0:8], wrc, h2[S][:, 0:8], start=False,
                           stop=(c == 7), skip_group_check=True)
                    gp[S] = g
                # fold L3 halves -> gf [128, 64]
                gf = [None, None]
                for S in range(2):
                    f = gfp.tile([128, 64], F32, tag=f"gf{'AB'[S]}")
                    nc.vector.tensor_reduce(
                        out=f[:],
                        in_=gp[S][:].rearrange("p (c pair j) -> p c j pair",
                                               c=8, pair=2),
                        axis=mybir.AxisListType.X, op=OP.add,
                    )
                    gf[S] = f
                # tanh -> ge[:, 0:64]
                for S in range(2):
                    nc.scalar.activation(ge_s[S][:, 0:64], gf[S][:], AF.Tanh,
                                         bias=0.0)
                # m = ge(c,j)*wf(c,j) in (j,c)-contiguous layout, then reduce
                q = [None, None]
                for S in range(2):
                    mt = mp.tile([128, 72], F32, tag=f"m{'AB'[S]}")
                    in0 = apv(ge_s[S][:], 0, [[1, 8], [8, 9]])
                    in1 = apv(wf[:], 8 * S, [[1, 8], [16, 9]])
                    outm = apv(mt[:], 0, [[9, 8], [1, 9]])
                    nc.gpsimd.tensor_tensor(out=outm, in0=in0, in1=in1,
                                            op=OP.mult)
                    qt = qp.tile([128, 8], F32, tag=f"q{'AB'[S]}")
                    nc.vector.tensor_reduce(
                        out=qt[:], in_=mt[:].rearrange("p (j c) -> p j c", c=9),
                        axis=mybir.AxisListType.X, op=OP.add,
                    )
                    q[S] = qt
                    qs[S].append(qt)

                # --- state updates ---
                if s < 3:
                    for S in range(2):
                        ge_n = gep.tile([128, 72], F32, tag=f"ge{S}")
                        zs_n = zp.tile([128, 16], F16, tag=f"zs{S}")
                        slot_cur = ge_cur[S][:, 64:72]
                        nc.vector.scalar_tensor_tensor(
                            out=ge_n[:, 64:72], in0=q[S][:],
                            scalar=stage_scale[s], in1=slot_cur,
                            op0=OP.mult, op1=OP.add,
                        )
                        nc.gpsimd.tensor_copy(out=zs_n[:, 0:8],
                                              in_=ge_n[:, 64:72])
                        nc.gpsimd.tensor_tensor(
                            out=zs_n[:, 8:16], in0=ge_n[:, 64:72],
                            in1=zs_n[:, 0:8], op=OP.subtract,
                        )
                        ge_s[S] = ge_n
                        zs_s[S] = zs_n
                if s == 1:
                    for S in range(2):
                        k = kp.tile([128, 8], F32, tag=f"k{'AB'[S]}")
                        nc.vector.scalar_tensor_tensor(
                            out=k[:], in0=qs[S][1][:], scalar=2.0,
                            in1=qs[S][0][:], op0=OP.mult, op1=OP.add,
                        )
                        kacc12[S] = k
                elif s == 2:
                    for S in range(2):
                        k = kp.tile([128, 8], F32, tag=f"k{'AB'[S]}")
                        nc.vector.scalar_tensor_tensor(
                            out=k[:], in0=qs[S][2][:], scalar=2.0,
                            in1=kacc12[S][:], op0=OP.mult, op1=OP.add,
                        )
                        kacc123[S] = k
                        pf = kp.tile([128, 8], F32, tag=f"pf{'AB'[S]}")
                        nc.vector.scalar_tensor_tensor(
                            out=pf[:], in0=k[:], scalar=1.0 / 6.0,
                            in1=ge_cur[S][:, 64:72], op0=OP.mult, op1=OP.add,
                        )
                        pfin[S] = pf
                elif s == 3:
                    for S in range(2):
                        ge_n = gep.tile([128, 72], F32, tag=f"ge{S}")
                        zs_n = zp.tile([128, 16], F16, tag=f"zs{S}")
                        nc.vector.scalar_tensor_tensor(
                            out=ge_n[:, 64:72], in0=q[S][:], scalar=1.0 / 6.0,
                            in1=pfin[S][:], op0=OP.mult, op1=OP.add,
                        )
                        nc.gpsimd.tensor_copy(out=zs_n[:, 0:8],
                                              in_=ge_n[:, 64:72])
                        nc.gpsimd.tensor_tensor(
                            out=zs_n[:, 8:16], in0=ge_n[:, 64:72],
                            in1=zs_n[:, 0:8], op=OP.subtract,
                        )
                        ge_next[S] = ge_n
                        zs_next[S] = zs_n
            ge_cur = ge_next
            zs_cur = zs_next

    nc.compile()
    return nc


def _split_f16(w):
    wh = np.asarray(w, np.float32).astype(np.float16)
    wr = (np.asarray(w, np.float32) - wh.astype(np.float32)).astype(np.float16)
    return wh, wr


def _prep_inputs(t, x, dyn_w1, dyn_b1, dyn_w2, dyn_b2, dyn_w3, dyn_b3,
                 init_w1, init_b1, init_w2, init_b2, init_w3, init_b3,
                 l_steps=L):
    x = np.asarray(x, dtype=np.float32)
    x_aug = np.concatenate([x, x[:, -1:]], axis=1)
    v = (x_aug[:, 1:] - x_aug[:, :-1]) / DT  # [B, L, X]
    sv = v.sum(-1)  # [B, L]

    w1s = np.concatenate([dyn_w1, dyn_w1], axis=0).astype(np.float32)
    w3x = np.empty((H, 1024), dtype=np.float32)
    for c in range(8):
        w3x[:, c * 128:(c + 1) * 128] = dyn_w3[:, ORIG_COL[c]]
    b3row = np.asarray(dyn_b3, np.float32)[ORIG_COL]  # [8, 128]

    b1h, b1r = _split_f16(np.asarray(dyn_b1, np.float32).reshape(1, 128))
    b2h, b2r = _split_f16(np.asarray(dyn_b2, np.float32).reshape(1, 128))
    biasw = np.zeros((2, 272), dtype=np.float16)
    biasw[0, 0:128] = b1h; biasw[1, 0:128] = b1r
    biasw[0, 128:256] = b2h; biasw[1, 128:256] = b2r
    biasw[:, 256:264] = 1.0
    w1h, w1r = _split_f16(w1s)
    w2h, w2r = _split_f16(dyn_w2)
    w3h, w3r = _split_f16(w3x)
    b3h, b3r = _split_f16(b3row)
    # seedW rows: 0:8 = b3h (row c -> chunk c), 8:16 = b3r
    seedW = np.concatenate([b3h, b3r], axis=0)  # [16, 128]
    # sel16[k, col]: col=(c,half,j): 1 iff (half=0,k=c) or (half=1,k=8+c)
    sel16 = np.zeros((16, 128), dtype=np.float16)
    for c in range(8):
        sel16[c, c * 16:c * 16 + 8] = 1.0
        sel16[8 + c, c * 16 + 8:c * 16 + 16] = 1.0

    wmm = np.concatenate([w1h, w1r, w2h, w2r, w3h, w3r], axis=1)  # [128, 2560]
    seed = np.concatenate([seedW, sel16], axis=1)                 # [16, 256]

    shared = dict(
        wmm=np.ascontiguousarray(wmm), seed=np.ascontiguousarray(seed),
        biasw=np.ascontiguousarray(biasw),
        wi2=np.asarray(init_w2, np.float32),
        wi3=np.asarray(init_w3, np.float32),
        bi1=np.asarray(init_b1, np.float32).reshape(128, 1),
        bi2=np.asarray(init_b2, np.float32).reshape(128, 1),
        bi3=np.asarray(init_b3, np.float32).reshape(64, 1),
    )
    wi1 = np.asarray(init_w1, np.float32)

    in_maps = []
    for core in range(NCORES):
        sl = slice(core * BPC, (core + 1) * BPC)
        vb = v[sl, :l_steps]            # [BPC, l, X]
        svb = sv[sl, :l_steps]          # [BPC, l]
        vsm = (DT * vb.transpose(1, 2, 0)).reshape(l_steps, 256).astype(np.float32)
        svdc = (-0.001 * DT * svb.T).astype(np.float32)  # [l, BPC]
        x0tc = x[sl, 0, :].T.astype(np.float32)          # [X, BPC]
        wi1x = np.concatenate([wi1, x0tc], axis=1)       # [16, 144]
        m = dict(shared)
        m.update(vsmall=np.ascontiguousarray(vsm), svd=np.ascontiguousarray(svdc),
                 wi1x=np.ascontiguousarray(wi1x))
        in_maps.append(m)
    return in_maps


_NC_CACHE = {}


def kernel_traced(trace=False, **inputs):
    key = L
    if key not in _NC_CACHE:
        _NC_CACHE[key] = build_nc(L)
    nc = _NC_CACHE[key]
    in_maps = _prep_inputs(**inputs, l_steps=L)
    res = run_bass_kernel_spmd(nc, in_maps, list(range(NCORES)), trace=trace)
    out = np.empty((B, L, Z), dtype=np.float32)
    for core in range(NCORES):
        for S, name in ((0, "zallA"), (1, "zallB")):
            zall = res.results[core][name]  # [L, 128, SPS] split form
            zf = zall[:, :Z] + zall[:, Z:]
            base = core * BPC + S * SPS
            out[base:base + SPS] = zf.transpose(2, 0, 1)
    return out, res


def kernel(**inputs):
    return kernel_traced(trace=False, **inputs)[0]
